# revision 1
# baseline (speedup 1.0000x reference)
"""ExpertBuffer fetch_on_demand: cache[slot_ids[k]] <- src[expert_ids[k]].

Pure scatter_memory problem. Sharding: slot-per-core expert parallelism —
core i owns cache slot i. The (slot, expert) index mapping is resolved on
the host (indices are host-visible numpy inputs), so each core's Bass
program is a pure DRAM->DRAM copy of the expert rows it needs:
  w13 row: (4096, 1024) f32 = 16 MiB
  w2  row: (1024, 2048) f32 =  8 MiB
  biases : 16 KiB + 4 KiB
Per-core HBM traffic: ~24 MiB read + ~24 MiB write.
"""

import numpy as np

import concourse.bass as bass
import concourse.mybir as mybir
from concourse.bass_utils import run_bass_kernel_spmd

N_CORES = 8
E_CACHE = 8
W13_SHAPE = [4096, 1024]
W13B_SHAPE = [4096]
W2_SHAPE = [1024, 2048]
W2B_SHAPE = [1024]

TENSORS = (
    ("w13", W13_SHAPE),
    ("w2", W2_SHAPE),
    ("w13b", W13B_SHAPE),
    ("w2b", W2B_SHAPE),
)

# BassKernelResults of the most recent kernel() call (for test harness use).
_LAST_RESULTS = None


def _build_program():
    nc = bass.Bass()
    f32 = mybir.dt.float32
    ins = {}
    outs = {}
    for name, shape in TENSORS:
        ins[name] = nc.declare_dram_parameter(f"{name}_in", shape, f32, isOutput=False)
    for name, shape in TENSORS:
        outs[name] = nc.declare_dram_parameter(f"{name}_out", shape, f32, isOutput=True)

    with nc.Block() as block, nc.semaphore("dma_sem") as dma_sem:

        @block.sync
        def _(sync):
            total = 0
            for name, _ in TENSORS:
                sync.dma_start(out=outs[name][:], in_=ins[name][:]).then_inc(
                    dma_sem, 16
                )
                total += 16
            sync.wait_ge(dma_sem, total)

    return nc


def kernel(
    w13_src,
    w13_bias_src,
    w2_src,
    w2_bias_src,
    w13_cache,
    w13_bias_cache,
    w2_cache,
    w2_bias_cache,
    expert_ids,
    slot_ids,
    _trace=False,
    _trace_cores=None,
):
    global _LAST_RESULTS

    w13_src = np.asarray(w13_src, dtype=np.float32)
    w13_bias_src = np.asarray(w13_bias_src, dtype=np.float32)
    w2_src = np.asarray(w2_src, dtype=np.float32)
    w2_bias_src = np.asarray(w2_bias_src, dtype=np.float32)
    w13_cache = np.asarray(w13_cache, dtype=np.float32)
    w13_bias_cache = np.asarray(w13_bias_cache, dtype=np.float32)
    w2_cache = np.asarray(w2_cache, dtype=np.float32)
    w2_bias_cache = np.asarray(w2_bias_cache, dtype=np.float32)
    eid = np.asarray(expert_ids).astype(np.int64)
    sid = np.asarray(slot_ids).astype(np.int64)

    # slot -> source expert, last write wins (scatter .at[].set semantics)
    row_expert = {}
    for k in range(sid.shape[0]):
        row_expert[int(sid[k])] = int(eid[k])

    in_maps = []
    for i in range(E_CACHE):
        if i in row_expert:
            e = row_expert[i]
            m = {
                "w13_in": w13_src[e],
                "w2_in": w2_src[e],
                "w13b_in": w13_bias_src[e],
                "w2b_in": w2_bias_src[e],
            }
        else:
            m = {
                "w13_in": w13_cache[i],
                "w2_in": w2_cache[i],
                "w13b_in": w13_bias_cache[i],
                "w2b_in": w2_bias_cache[i],
            }
        in_maps.append(m)

    nc = _build_program()
    res = run_bass_kernel_spmd(
        nc,
        in_maps,
        list(range(N_CORES)),
        trace=_trace,
        trace_cores=_trace_cores,
    )
    _LAST_RESULTS = res
    r = res.results

    w13_out = np.stack([r[i]["w13_out"] for i in range(E_CACHE)])
    w13b_out = np.stack([r[i]["w13b_out"] for i in range(E_CACHE)])
    w2_out = np.stack([r[i]["w2_out"] for i in range(E_CACHE)])
    w2b_out = np.stack([r[i]["w2b_out"] for i in range(E_CACHE)])
    return (w13_out, w13b_out, w2_out, w2b_out)


# revision 2
# speedup vs baseline: 1.1824x; 1.1824x over previous
"""ExpertBuffer fetch_on_demand: cache[slot_ids[k]] <- src[expert_ids[k]].

Pure scatter_memory problem. Sharding: slot-per-core expert parallelism —
core i owns cache slot i. The (slot, expert) index mapping is resolved on
the host (indices are host-visible numpy inputs), so each core's Bass
program is a pure DRAM->DRAM copy of the expert rows it needs:
  w13 row: (4096, 1024) f32 = 16 MiB
  w2  row: (1024, 2048) f32 =  8 MiB
  biases : 16 KiB + 4 KiB
Per-core HBM traffic: ~24 MiB read + ~24 MiB write.
"""

import numpy as np

import concourse.bass as bass
import concourse.mybir as mybir
from concourse.bass_utils import run_bass_kernel_spmd

N_CORES = 8
E_CACHE = 8
W13_SHAPE = [4096, 1024]
W13B_SHAPE = [4096]
W2_SHAPE = [1024, 2048]
W2B_SHAPE = [1024]

TENSORS = (
    ("w13", W13_SHAPE),
    ("w2", W2_SHAPE),
    ("w13b", W13B_SHAPE),
    ("w2b", W2B_SHAPE),
)

# BassKernelResults of the most recent kernel() call (for test harness use).
_LAST_RESULTS = None


def _build_program():
    nc = bass.Bass()
    f32 = mybir.dt.float32
    ins = {}
    outs = {}
    for name, shape in TENSORS:
        ins[name] = nc.declare_dram_parameter(f"{name}_in", shape, f32, isOutput=False)
    for name, shape in TENSORS:
        outs[name] = nc.declare_dram_parameter(f"{name}_out", shape, f32, isOutput=True)

    # Two HWDGE rings (sync=SP, scalar=ACT) each stream half of every big
    # tensor; the 16 SDMA engines round-robin both rings' packets, doubling
    # in-flight depth to hide HBM latency. Biases ride on sync.
    h13 = W13_SHAPE[0] // 2
    h2 = W2_SHAPE[0] // 2
    sync_copies = [
        (outs["w13"][:h13], ins["w13"][:h13]),
        (outs["w2"][:h2], ins["w2"][:h2]),
        (outs["w13b"][:], ins["w13b"][:]),
        (outs["w2b"][:], ins["w2b"][:]),
    ]
    scalar_copies = [
        (outs["w13"][h13:], ins["w13"][h13:]),
        (outs["w2"][h2:], ins["w2"][h2:]),
    ]
    total = 16 * (len(sync_copies) + len(scalar_copies))

    with nc.Block() as block, nc.semaphore("dma_sem") as dma_sem:

        @block.scalar
        def _(scalar):
            for out, in_ in scalar_copies:
                scalar.dma_start(out=out, in_=in_).then_inc(dma_sem, 16)

        @block.sync
        def _(sync):
            for out, in_ in sync_copies:
                sync.dma_start(out=out, in_=in_).then_inc(dma_sem, 16)
            sync.wait_ge(dma_sem, total)

    return nc


def kernel(
    w13_src,
    w13_bias_src,
    w2_src,
    w2_bias_src,
    w13_cache,
    w13_bias_cache,
    w2_cache,
    w2_bias_cache,
    expert_ids,
    slot_ids,
    _trace=False,
    _trace_cores=None,
):
    global _LAST_RESULTS

    w13_src = np.asarray(w13_src, dtype=np.float32)
    w13_bias_src = np.asarray(w13_bias_src, dtype=np.float32)
    w2_src = np.asarray(w2_src, dtype=np.float32)
    w2_bias_src = np.asarray(w2_bias_src, dtype=np.float32)
    w13_cache = np.asarray(w13_cache, dtype=np.float32)
    w13_bias_cache = np.asarray(w13_bias_cache, dtype=np.float32)
    w2_cache = np.asarray(w2_cache, dtype=np.float32)
    w2_bias_cache = np.asarray(w2_bias_cache, dtype=np.float32)
    eid = np.asarray(expert_ids).astype(np.int64)
    sid = np.asarray(slot_ids).astype(np.int64)

    # slot -> source expert, last write wins (scatter .at[].set semantics)
    row_expert = {}
    for k in range(sid.shape[0]):
        row_expert[int(sid[k])] = int(eid[k])

    in_maps = []
    for i in range(E_CACHE):
        if i in row_expert:
            e = row_expert[i]
            m = {
                "w13_in": w13_src[e],
                "w2_in": w2_src[e],
                "w13b_in": w13_bias_src[e],
                "w2b_in": w2_bias_src[e],
            }
        else:
            m = {
                "w13_in": w13_cache[i],
                "w2_in": w2_cache[i],
                "w13b_in": w13_bias_cache[i],
                "w2b_in": w2_bias_cache[i],
            }
        in_maps.append(m)

    nc = _build_program()
    res = run_bass_kernel_spmd(
        nc,
        in_maps,
        list(range(N_CORES)),
        trace=_trace,
        trace_cores=_trace_cores,
    )
    _LAST_RESULTS = res
    r = res.results

    w13_out = np.stack([r[i]["w13_out"] for i in range(E_CACHE)])
    w13b_out = np.stack([r[i]["w13b_out"] for i in range(E_CACHE)])
    w2_out = np.stack([r[i]["w2_out"] for i in range(E_CACHE)])
    w2b_out = np.stack([r[i]["w2b_out"] for i in range(E_CACHE)])
    return (w13_out, w13b_out, w2_out, w2b_out)
